# revision 1
# baseline (speedup 1.0000x reference)
"""BiLSTM Trainium2 kernel (8 NeuronCores, SPMD).

Problem: inputs [64, 512, 256] f32, BiLSTM hidden 512, out = (fwd + bwd)/2.

Sharding: 8 cores = 4 batch shards (16 each) x 2 directions. Backward cores
receive time-reversed inputs and W_b; the program is identical (SPMD).

Per-core program:
  Phase A: px = [x|1] @ [Wx;b]  (gate-permuted columns), fp16 token-tiled
           GEMM (M=128 tokens), spilled to DRAM as fp16.
  Phase B: 512 recurrent steps. Gates PSUM bank laid out [128p, 512f]:
           partition 32*j+b (j = h-block, b = batch), free 128*g+k (g = gate
           f/i/o/C, k = h-dim within block). Per step:
             inject px via identity matmul (4 col-tiled MMs),
             h-GEMM: 4 K-chunks x 4 col-groups fp16 col-tiled MMs,
             sigmoid(f,i,o) one ACT op [128,384], tanh(C) [128,128],
             c' = sf*c + si*tg (GpSimd + 2 DVE), tanh(c') ACT,
             h = so*tanh(c') DVE, 4 PE transposes -> hT fp16 stationary.
Weights column-permuted: new col 512*j + 128*g + k  <- orig col 512*g + 128*j + k.
"""
import sys
sys.path.insert(0, "/opt/trn_rl_repo")
import numpy as np

import concourse.bacc as bacc
import concourse.tile as tile
from concourse import mybir

F32 = mybir.dt.float32
FP16 = mybir.dt.float16
SIG = mybir.ActivationFunctionType.Sigmoid
TANH = mybir.ActivationFunctionType.Tanh
MUL = mybir.AluOpType.mult

I_SIZE, H_SIZE = 256, 512
B_FULL, S_FULL = 64, 512
N_CORES = 8
BL = 16                      # batch rows per core


def _perm_cols():
    """new col' = 512*j + 128*g + k  maps from orig col = 512*g + 128*j + k."""
    p = np.empty(4 * H_SIZE, dtype=np.int64)
    for j in range(4):
        for g in range(4):
            for k in range(128):
                p[512 * j + 128 * g + k] = 512 * g + 128 * j + k
    return p


def build_program(S=S_FULL):
    """Build the per-core SPMD program. Returns compiled nc."""
    assert S % 8 == 0
    NT = S * BL // 128       # token tiles in phase A

    nc = bacc.Bacc("TRN2", target_bir_lowering=False, debug=False)

    d_xT = nc.dram_tensor("xT", [2, 128, S * BL], FP16, kind="ExternalInput").ap()
    d_Wx = nc.dram_tensor("Wx", [2, 128, 2048], FP16, kind="ExternalInput").ap()
    d_bias = nc.dram_tensor("bias", [1, 2048], FP16, kind="ExternalInput").ap()
    d_Wh = nc.dram_tensor("Wh", [4, 128, 2048], FP16, kind="ExternalInput").ap()
    d_eyei = nc.dram_tensor("eyei", [4, 16, 128], FP16, kind="ExternalInput").ap()
    d_eyeT = nc.dram_tensor("eyeT", [128, 128], F32, kind="ExternalInput").ap()
    d_y = nc.dram_tensor("y", [S, 128, 128], F32, kind="ExternalOutput").ap()

    with tile.TileContext(nc) as tc:
        with tc.tile_pool(name="persist", bufs=1) as pers, \
             tc.tile_pool(name="dram", bufs=1, space="DRAM") as dram:
            wh_sb = pers.tile([128, 4, 2048], FP16, tag="wh")
            for c in range(4):
                nc.sync.dma_start(wh_sb[:, c, :], d_Wh[c, :, :])
            eyei_sb = pers.tile([16, 4, 128], FP16, tag="eyei")
            for j in range(4):
                nc.sync.dma_start(eyei_sb[:, j, :], d_eyei[j, :, :])
            eyeT_sb = pers.tile([128, 128], F32, tag="eyeT")
            nc.sync.dma_start(eyeT_sb[:], d_eyeT)

            px_dram = dram.tile([NT, 128, 2048], FP16, tag="px")

            # ---------------- Phase A: px precompute ----------------
            with tc.tile_pool(name="pa_sb", bufs=1) as pa, \
                 tc.tile_pool(name="pa_px", bufs=2) as pa_px, \
                 tc.tile_pool(name="pa_ps", bufs=2, space="PSUM") as pa_ps:
                xT_sb = pa.tile([128, 2, S * BL], FP16, tag="xT")
                for c in range(2):
                    nc.sync.dma_start(xT_sb[:, c, :], d_xT[c, :, :])
                wx_sb = pa.tile([128, 2, 2048], FP16, tag="wx")
                for c in range(2):
                    nc.sync.dma_start(wx_sb[:, c, :], d_Wx[c, :, :])
                bias_sb = pa.tile([1, 2048], FP16, tag="bias")
                nc.sync.dma_start(bias_sb[:], d_bias)
                ones_sb = pa.tile([1, 128], FP16, tag="ones")
                nc.vector.memset(ones_sb[:], 1.0)

                for jj in range(NT):
                    pxp = pa_ps.tile([128, 2048], F32, tag="pxp")
                    for nt in range(4):
                        for c in range(2):
                            nc.tensor.matmul(
                                pxp[:, 512 * nt:512 * (nt + 1)],
                                xT_sb[:, c, 128 * jj:128 * (jj + 1)],
                                wx_sb[:, c, 512 * nt:512 * (nt + 1)],
                                start=(c == 0), stop=False, skip_group_check=True)
                        nc.tensor.matmul(
                            pxp[:, 512 * nt:512 * (nt + 1)],
                            ones_sb[0:1, :],
                            bias_sb[0:1, 512 * nt:512 * (nt + 1)],
                            start=False, stop=True, skip_group_check=True)
                    pxs = pa_px.tile([128, 2048], FP16, tag="pxs")
                    nc.scalar.copy(pxs[:, 0:1024], pxp[:, 0:1024])
                    nc.vector.tensor_copy(pxs[:, 1024:2048], pxp[:, 1024:2048])
                    nc.sync.dma_start(px_dram[jj], pxs[:])

            # ---------------- Phase B: recurrence ----------------
            with tc.tile_pool(name="pb_sb", bufs=2) as pb, \
                 tc.tile_pool(name="pb_pp", bufs=1) as pp, \
                 tc.tile_pool(name="pb_stage", bufs=4) as pb_st, \
                 tc.tile_pool(name="pb_ps", bufs=1, space="PSUM") as pb_ps:

                # persistent ping-pong tiles (manual slots; zero-padding persists)
                gates_pp = [pb_ps.tile([128, 512], F32, tag=f"g{i}", name=f"g{i}") for i in (0, 1)]
                tr_pp = [pb_ps.tile([128, 128], F32, tag=f"tr{i}", name=f"tr{i}") for i in (0, 1)]
                hT_pp = [pp.tile([128, 128], FP16, tag=f"hT{i}", name=f"hT{i}") for i in (0, 1)]
                nc.vector.memset(hT_pp[0][:], 0.0)
                nc.vector.memset(hT_pp[1][:], 0.0)
                c_prev = pb.tile([128, 128], F32, tag="c")
                nc.vector.memset(c_prev[:], 0.0)

                def stage_dma(t):
                    stg = pb_st.tile([16, 2048], FP16, tag="stg")
                    m = t % 8
                    nc.sync.dma_start(stg[:], px_dram[t // 8, 16 * m:16 * (m + 1), :])
                    return stg

                def inject(t, stg):
                    # 4 full-M matmuls: eyei[:, j, 32*j + b] = 1 routes px rows to
                    # partition group j; zero columns accumulate as no-ops.
                    g = gates_pp[t % 2]
                    for j in range(4):
                        nc.tensor.matmul(
                            g[:, :],
                            eyei_sb[:, j, :],
                            stg[:, 512 * j:512 * (j + 1)],
                            start=(j == 0), stop=False, skip_group_check=True)
                    return g

                stg = stage_dma(0)
                gates = inject(0, stg)
                nxt_stg = stage_dma(1) if S > 1 else None
                for t in range(S):
                    hT_prev = hT_pp[(t + 1) % 2]
                    # h-GEMM: 4 K-chunks x 4 col groups (M=32, cols 16-31 zero)
                    for c in range(4):
                        for j in range(4):
                            nc.tensor.matmul(
                                gates[32 * j:32 * (j + 1), :],
                                hT_prev[:, 32 * c:32 * (c + 1)],
                                wh_sb[:, c, 512 * j:512 * (j + 1)],
                                start=False, stop=(c == 3), skip_group_check=True,
                                tile_position=(0, 32 * j))
                    # activations on gate bank
                    sg = pb.tile([128, 384], F32, tag="sg")
                    nc.scalar.activation(sg[:], gates[:, 0:384], SIG)
                    tg = pb.tile([128, 128], F32, tag="tg")
                    nc.scalar.activation(tg[:], gates[:, 384:512], TANH)
                    # c' = sf*c + si*tg
                    m_t = pb.tile([128, 128], F32, tag="m")
                    nc.gpsimd.tensor_tensor(m_t[:], sg[:, 0:128], c_prev[:], MUL)
                    t1 = pb.tile([128, 128], F32, tag="t1")
                    nc.vector.tensor_mul(t1[:], sg[:, 128:256], tg[:])
                    c_new = pb.tile([128, 128], F32, tag="c")
                    nc.vector.tensor_add(c_new[:], m_t[:], t1[:])
                    # h = so * tanh(c')
                    tc_t = pb.tile([128, 128], F32, tag="tc")
                    nc.scalar.activation(tc_t[:], c_new[:], TANH)
                    h_t = pb.tile([128, 128], F32, tag="h")
                    nc.vector.tensor_mul(h_t[:], sg[:, 256:384], tc_t[:])

                    if t + 1 < S:
                        gates = inject(t + 1, nxt_stg)
                        if t + 2 < S:
                            nxt_stg = stage_dma(t + 2)
                        # transpose h -> hT (fp16 stationary for next step)
                        ptr = tr_pp[t % 2]
                        nc.tensor.transpose(ptr[:], h_t[:], eyeT_sb[:])
                        nc.scalar.copy(hT_pp[t % 2][:], ptr[:])
                    c_prev = c_new
                    nc.gpsimd.dma_start(d_y[t], h_t[:])

    nc.compile()
    return nc


def prep_core_inputs(x_slice, W, b, reverse):
    """Host-side prep of one core's input map.

    x_slice: [BL, S, I] f32; W: [768, 2048]; b: [2048]; reverse: bwd dir.
    """
    S = x_slice.shape[1]
    perm = _perm_cols()
    xx = x_slice[:, ::-1, :] if reverse else x_slice
    xT = np.ascontiguousarray(xx.transpose(2, 1, 0)).reshape(2, 128, S * BL)
    Wp = W[:, perm]
    Wx = np.ascontiguousarray(Wp[:I_SIZE]).reshape(2, 128, 2048)
    Wh = np.ascontiguousarray(Wp[I_SIZE:]).reshape(4, 128, 2048)
    bias = b[perm].reshape(1, 2048)
    eyeT = np.eye(128, dtype=np.float32)
    eyei = np.zeros((4, 16, 128), np.float16)
    for j in range(4):
        for b_ in range(16):
            eyei[j, b_, 32 * j + b_] = 1
    return {
        "xT": xT.astype(np.float16),
        "Wx": Wx.astype(np.float16),
        "bias": bias.astype(np.float16),
        "Wh": Wh.astype(np.float16),
        "eyei": eyei,
        "eyeT": eyeT,
    }


def assemble_output(results, S=S_FULL, B=B_FULL):
    """results: list of 8 per-core out dicts with 'y' [S,128,128] f32."""
    out_f = np.empty((B, S, H_SIZE), np.float32)
    out_b = np.empty((B, S, H_SIZE), np.float32)
    for core in range(N_CORES):
        yc = results[core]["y"]                       # [S, 128, 128]
        hc = yc.reshape(S, 4, 32, 128)[:, :, :BL, :]  # [S, 4, 16, 128]
        hc = np.ascontiguousarray(hc.transpose(2, 0, 1, 3)).reshape(BL, S, H_SIZE)
        if core < 4:
            out_f[BL * core:BL * (core + 1)] = hc
        else:
            cc = core - 4
            out_b[BL * cc:BL * (cc + 1)] = hc[:, ::-1, :]
    return (out_f + out_b) * 0.5


def make_in_maps(inputs, W_f, b_f, W_b, b_b, S=S_FULL):
    in_maps = []
    for core in range(N_CORES):
        if core < 4:
            sl = inputs[BL * core:BL * (core + 1), :S]
            in_maps.append(prep_core_inputs(sl, W_f, b_f, reverse=False))
        else:
            cc = core - 4
            sl = inputs[BL * cc:BL * (cc + 1), :S]
            in_maps.append(prep_core_inputs(sl, W_b, b_b, reverse=True))
    return in_maps


_NC_CACHE = {}


def kernel(inputs, W_f, b_f, W_b, b_b):
    from concourse.bass_utils import run_bass_kernel_spmd
    inputs = np.asarray(inputs, dtype=np.float32)
    S = inputs.shape[1]
    if S not in _NC_CACHE:
        _NC_CACHE[S] = build_program(S)
    nc = _NC_CACHE[S]
    in_maps = make_in_maps(inputs, np.asarray(W_f), np.asarray(b_f),
                           np.asarray(W_b), np.asarray(b_b), S)
    res = run_bass_kernel_spmd(nc, in_maps, core_ids=list(range(N_CORES)))
    return assemble_output(res.results, S, inputs.shape[0])

